# revision 7
# baseline (speedup 1.0000x reference)
"""CaptioningRNN (LSTM + tiny spatial attention) Trainium2 kernel, v2.

Contract: kernel(**inputs) takes FULL inputs (numpy), returns FULL output
(N, T, H) float32.  Data-parallel over batch N across 8 NeuronCores.

v2 design notes (per-core, NL=16 sequences):
  - All sigmoids become tanh via sig(x) = (tanh(x/2)+1)/2 with the 1/2
    factors folded into pre-scaled weights; together with a doubled state
    hh = 2h, the whole gate nonlinearity is ONE ACT op (single activation
    table set -> no per-step table loads).
  - Gate matmuls are col-tiled (tile_position), gate fanned to PE column
    group g in {i:0,f:1,o:2,g:3}; psa is a single [128,512] PSUM bank.
    Stationary operands padded to 32 cols so every PSUM row is written.
  - One dma_start_transpose moves tanh(gates) [128,512] -> [128,4,128]
    (h-on-partitions), so the LSTM tail runs at FD=64 and h emerges
    already transposed for the next step; no PE transposes.
  - Attention keeps two copies of A ((m,n)- and (n,m)-ordered) so both
    big DVE ops run in 2x packed mode; softmax normalization is deferred
    to the attn-transpose step; broadcasts via gpsimd.partition_broadcast.
  - cell state s = 2c kept f32 transposed; h output written transposed,
    fixed up (transpose + *0.5) on host.
"""

import sys
import numpy as np

sys.path.insert(0, "/opt/trn_rl_repo")

import ml_dtypes

BF16 = ml_dtypes.bfloat16

N, T, D, H, M = 128, 512, 512, 512, 16
NCORES = 8
NL = N // NCORES          # 16 sequences per core
KC = 4                    # 512 = 4 chunks of 128 (contraction dims)
J = 4 * H                 # 2048 gate columns
TB = 8                    # time steps per phase-0 row block
RB = NL * T // 128        # phase-0 row blocks

_CACHE = {}


def build(t_steps=T, has_bias=False):
    from concourse import bacc, mybir
    import concourse.tile as tile

    f32 = mybir.dt.float32
    bf16 = mybir.dt.bfloat16
    mult = mybir.AluOpType.mult
    add = mybir.AluOpType.add
    AF = mybir.ActivationFunctionType
    AX = mybir.AxisListType.X

    rb = NL * t_steps // 128

    nc = bacc.Bacc("TRN2", target_bir_lowering=False, debug=False,
                   num_devices=NCORES)

    # ---- I/O -----------------------------------------------------------
    xs = nc.dram_tensor("xs", [rb, 128, KC, 128], bf16, kind="ExternalInput")
    atmn_d = nc.dram_tensor("atmn", [128, KC, M, NL], bf16, kind="ExternalInput")
    atnm_d = nc.dram_tensor("atnm", [128, KC, NL, M], bf16, kind="ExternalInput")
    wx_d = nc.dram_tensor("wx", [128, KC, J], bf16, kind="ExternalInput")
    wh_d = nc.dram_tensor("wh", [128, KC, J], bf16, kind="ExternalInput")
    wa_d = nc.dram_tensor("wa", [128, KC, J], bf16, kind="ExternalInput")
    hh0_d = nc.dram_tensor("hh0", [128, KC, 32], bf16, kind="ExternalInput")
    s0_d = nc.dram_tensor("s0", [128, KC, NL], f32, kind="ExternalInput")
    id32_d = nc.dram_tensor("id32", [NL, 32], bf16, kind="ExternalInput")
    oc_d = nc.dram_tensor("ones_col", [128, 1], bf16, kind="ExternalInput")
    if has_bias:
        b_d = nc.dram_tensor("bvec", [1, J], f32, kind="ExternalInput")
        or_d = nc.dram_tensor("ones_row", [1, 128], bf16, kind="ExternalInput")
    p_d = nc.dram_tensor("pbuf", [rb, 128, J], bf16)
    out_d = nc.dram_tensor("outT", [t_steps, 128, KC * NL], bf16,
                           kind="ExternalOutput")

    inv2 = float(0.5 / np.sqrt(H))   # exp scale: inv_sqrt_h / 2 (hh = 2h)

    from contextlib import ExitStack
    with tile.TileContext(nc) as tc, ExitStack() as stack:
        # ---- persistent constants -------------------------------------
        cpool = stack.enter_context(tc.tile_pool(name="consts", bufs=1))
        wh_s = cpool.tile([128, KC, J], bf16)
        wa_s = cpool.tile([128, KC, J], bf16)
        atmn = cpool.tile([128, KC, M, NL], bf16)
        atnm = cpool.tile([128, KC, NL, M], bf16)
        id32 = cpool.tile([NL, 32], bf16)
        oc_s = cpool.tile([128, 1], bf16)
        nc.sync.dma_start(out=wh_s[:, :, :], in_=wh_d.ap()[:, :, :])
        nc.sync.dma_start(out=wa_s[:, :, :], in_=wa_d.ap()[:, :, :])
        nc.sync.dma_start(out=atmn[:, :, :, :], in_=atmn_d.ap()[:, :, :, :])
        nc.sync.dma_start(out=atnm[:, :, :, :], in_=atnm_d.ap()[:, :, :, :])
        nc.sync.dma_start(out=id32[:, :], in_=id32_d.ap()[:, :])
        nc.sync.dma_start(out=oc_s[:, :], in_=oc_d.ap()[:, :])

        # ---- phase 0: P = x @ Wx' (+ b') ------------------------------
        with tc.tile_pool(name="ph0", bufs=1) as p0c, \
             tc.tile_pool(name="ph0x", bufs=3) as p0x, \
             tc.tile_pool(name="ph0o", bufs=3) as p0o, \
             tc.tile_pool(name="ps0", bufs=2, space="PSUM") as ps0:
            wx_s = p0c.tile([128, KC, J], bf16)
            nc.sync.dma_start(out=wx_s[:, :, :], in_=wx_d.ap()[:, :, :])
            if has_bias:
                bf_s = p0c.tile([1, J], f32)
                nc.sync.dma_start(out=bf_s[:, :], in_=b_d.ap()[:, :])
                bb_s = p0c.tile([1, J], bf16)
                nc.vector.tensor_copy(bb_s[:, :], bf_s[:, :])
                brep = p0c.tile([128, J], bf16)
                nc.gpsimd.partition_broadcast(brep[:, :], bb_s[:, :])

            for b_i in range(rb):
                xt = p0x.tile([128, KC, 128], bf16, tag="xt")
                nc.sync.dma_start(out=xt[:, :, :], in_=xs.ap()[b_i, :, :, :])
                psp = ps0.tile([128, J], f32, tag="psp")
                for kc in range(KC):
                    for jt in range(4):
                        nc.tensor.matmul(
                            psp[:, jt * 512:(jt + 1) * 512],
                            xt[:, kc, :],
                            wx_s[:, kc, jt * 512:(jt + 1) * 512],
                            start=(kc == 0), stop=(kc == KC - 1))
                pout = p0o.tile([128, J], bf16, tag="pout")
                for jt in range(4):
                    sl = slice(jt * 512, (jt + 1) * 512)
                    if has_bias:
                        nc.vector.tensor_tensor(pout[:, sl], psp[:, sl],
                                                brep[:, sl], add)
                    elif jt in (1, 3):
                        nc.scalar.copy(pout[:, sl], psp[:, sl])
                    else:
                        nc.vector.tensor_copy(pout[:, sl], psp[:, sl])
                nc.sync.dma_start(out=p_d.ap()[b_i, :, :], in_=pout[:, :])

        # ---- phase 1: recurrence --------------------------------------
        with tc.tile_pool(name="state", bufs=1) as stp, \
             tc.tile_pool(name="work", bufs=2) as wk, \
             tc.tile_pool(name="pin", bufs=3) as pin, \
             tc.tile_pool(name="tout", bufs=3) as top, \
             tc.tile_pool(name="ps_a", bufs=2, space="PSUM") as psa_p, \
             tc.tile_pool(name="ps_s", bufs=2, space="PSUM") as pss:

            # explicit double-buffered state
            hh = [stp.tile([128, KC, 32], bf16, tag=f"hh{i}", name=f"hh{i}")
                  for i in (0, 1)]
            sT = [stp.tile([128, KC, NL], f32, tag=f"sT{i}", name=f"sT{i}")
                  for i in (0, 1)]
            aT = stp.tile([128, KC, 32], bf16, tag="aT")
            nc.sync.dma_start(out=hh[0][:, :, :], in_=hh0_d.ap()[:, :, :])
            nc.sync.dma_start(out=sT[0][:, :, :], in_=s0_d.ap()[:, :, :])
            nc.vector.memset(hh[1][:, :, 16:32], 0.0)
            nc.vector.memset(aT[:, :, 16:32], 0.0)

            for t in range(t_steps):
                cur, nxt = hh[t % 2], hh[(t + 1) % 2]
                scur, snxt = sT[t % 2], sT[(t + 1) % 2]
                b_i, tt = divmod(t, TB)

                p_t = pin.tile([NL, J], bf16, tag="pt")
                nc.sync.dma_start(out=p_t[:, :],
                                  in_=p_d.ap()[b_i, tt * NL:(tt + 1) * NL, :])

                # -- gates part 1: P inject + hh@Wh (col-tiled) ---------
                psa = psa_p.tile([128, 512], f32, tag="a")
                for g in range(4):
                    nc.tensor.matmul(psa[32 * g:32 * g + 32, :],
                                     id32[:, :],
                                     p_t[:, g * 512:(g + 1) * 512],
                                     start=True, stop=False,
                                     skip_group_check=True,
                                     tile_position=(0, 32 * g))
                for kc in range(KC):
                    for g in range(4):
                        nc.tensor.matmul(
                            psa[32 * g:32 * g + 32, :],
                            cur[:, kc, :],
                            wh_s[:, kc, g * 512:(g + 1) * 512],
                            start=False, stop=False,
                            skip_group_check=True,
                            tile_position=(0, 32 * g))

                # -- attention ------------------------------------------
                # s2[p,kc,m,n] = A[p,kc,m,n] * hh[p,kc,n]
                s2 = wk.tile([128, KC, M, NL], bf16, tag="s2")
                nc.vector.tensor_tensor(
                    s2[:, :, :, :], atmn[:, :, :, :],
                    cur[:, :, None, 0:16].broadcast_to([128, KC, M, NL]),
                    mult)
                psz = pss.tile([1, M, NL], f32, tag="z")
                for kc in range(KC):
                    nc.tensor.matmul(psz[:, :, :], oc_s[:, :], s2[:, kc, :, :],
                                     start=(kc == 0), stop=(kc == KC - 1))
                # e[n,m] = exp(scores/sqrt(H)); write (n,m)-ordered
                e_sb = wk.tile([1, NL, M], bf16, tag="e")
                nc.scalar.activation(
                    e_sb[:, :, :].rearrange("p n m -> p m n"),
                    psz[:, :, :], AF.Exp, scale=inv2)
                sum_e = wk.tile([1, NL, 1], f32, tag="sume")
                nc.vector.tensor_reduce(sum_e[:, :, :], e_sb[:, :, :], AX, add)
                rec = wk.tile([1, NL, 1], f32, tag="rec")
                nc.vector.reciprocal(rec[:, :, :], sum_e[:, :, :])
                # broadcast e (n,m) and 1/sum_e across partitions (gpsimd)
                w_b = wk.tile([128, NL, M], bf16, tag="wb")
                nc.gpsimd.partition_broadcast(w_b[:, :, :], e_sb[:, :, :])
                recB = wk.tile([128, NL], f32, tag="recB")
                nc.gpsimd.partition_broadcast(recB[:, :], rec[:, :, 0])
                # attnU[p,kc,n] = sum_m A[p,kc,n,m] * e[n,m]
                p2 = wk.tile([128, KC, NL, M], bf16, tag="p2")
                nc.vector.tensor_tensor(
                    p2[:, :, :, :], atnm[:, :, :, :],
                    w_b[:, None, :, :].broadcast_to([128, KC, NL, M]), mult)
                attnU = wk.tile([128, KC, NL], bf16, tag="attnU")
                with nc.allow_low_precision(reason="16-wide sum; 0.4% ok"):
                    nc.vector.tensor_reduce(attnU[:, :, :, None],
                                            p2[:, :, :, :], AX, add)
                nc.vector.tensor_tensor(
                    aT[:, :, 0:16], attnU[:, :, :],
                    recB[:, None, :].broadcast_to([128, KC, NL]), mult)

                # -- gates part 2: attn@Wattn (col-tiled) ---------------
                for kc in range(KC):
                    for g in range(4):
                        nc.tensor.matmul(
                            psa[32 * g:32 * g + 32, :],
                            aT[:, kc, :],
                            wa_s[:, kc, g * 512:(g + 1) * 512],
                            start=False, stop=(kc == KC - 1),
                            skip_group_check=True,
                            tile_position=(0, 32 * g))

                # -- nonlinearity + transpose ---------------------------
                t_all = wk.tile([128, 512], bf16, tag="tall")
                nc.scalar.activation(t_all[:, :], psa[:, :], AF.Tanh)
                tT = wk.tile([128, KC, 128], bf16, tag="tT")
                nc.sync.dma_start_transpose(tT[:, :, :], t_all[:, :])
                # gate slices in transposed layout: rows 32g+n
                tnh_i = tT[:, :, 0:16]
                tnh_f = tT[:, :, 32:48]
                tnh_o = tT[:, :, 64:80]
                tnh_g = tT[:, :, 96:112]

                # -- tail: s' = 0.5*(tanh_f+1)*s + (tanh_i+1)*tanh_g ----
                tA = wk.tile([128, KC, NL], f32, tag="tA")
                nc.vector.scalar_tensor_tensor(
                    out=tA[:, :, :], in0=tnh_f, scalar=1.0,
                    in1=scur[:, :, :], op0=add, op1=mult)
                tB = wk.tile([128, KC, NL], f32, tag="tB")
                nc.vector.scalar_tensor_tensor(
                    out=tB[:, :, :], in0=tnh_i, scalar=1.0,
                    in1=tnh_g, op0=add, op1=mult)
                nc.vector.scalar_tensor_tensor(
                    out=snxt[:, :, :], in0=tA[:, :, :], scalar=0.5,
                    in1=tB[:, :, :], op0=mult, op1=add)
                tanh_c = wk.tile([128, KC, NL], bf16, tag="tc")
                nc.scalar.activation(tanh_c[:, :, :], snxt[:, :, :], AF.Tanh,
                                     scale=0.5)
                nc.vector.scalar_tensor_tensor(
                    out=nxt[:, :, 0:16], in0=tnh_o, scalar=1.0,
                    in1=tanh_c[:, :, :], op0=add, op1=mult)

                # -- output (transposed; host fixes up) -----------------
                ho = top.tile([128, KC, NL], bf16, tag="ho")
                nc.vector.tensor_copy(ho[:, :, :], nxt[:, :, 0:16])
                nc.sync.dma_start(out=out_d.ap()[t, :, :],
                                  in_=ho[:, :, :].rearrange("p a b -> p (a b)"))

    nc.compile()
    return nc


def _stage_inputs(x, A, Wx, Wh, Wattn, b, t_steps=T):
    """Shard + lay out inputs per core (host-side numpy staging)."""
    rb = NL * t_steps // 128
    h0 = A.mean(axis=(2, 3)).astype(np.float32)          # (N, H)
    id32 = np.concatenate([np.eye(NL, dtype=BF16),
                           np.zeros((NL, NL), dtype=BF16)], axis=1)
    ones_col = np.ones((128, 1), dtype=BF16)

    # fold tanh-trick scales into weights: i,f,o gate columns * 0.5
    iof = np.ones((J,), dtype=np.float32)
    iof[:3 * H] = 0.5
    # hh = 2h -> Wh gets an extra 0.5
    Wx_e = Wx.astype(np.float32) * iof
    Wh_e = Wh.astype(np.float32) * iof * 0.5
    Wa_e = Wattn.astype(np.float32) * iof
    b_e = b.astype(np.float32) * iof

    def wlay(w):
        return np.ascontiguousarray(
            w.astype(BF16).reshape(KC, 128, J).transpose(1, 0, 2))

    wxs, whs, was = wlay(Wx_e), wlay(Wh_e), wlay(Wa_e)
    bvec = np.ascontiguousarray(b_e.reshape(1, J))

    maps = []
    for k in range(NCORES):
        ns = slice(k * NL, (k + 1) * NL)
        x_sh = x[ns, :t_steps].astype(BF16)              # (NL, t, D)
        xT = x_sh.transpose(2, 0, 1).reshape(KC, 128, NL, rb, TB)
        xs_st = np.ascontiguousarray(
            xT.transpose(3, 1, 0, 4, 2).reshape(rb, 128, KC, 128))
        A_sh = A[ns].reshape(NL, H, M).astype(np.float32)  # (n, h, m)
        # at[p, kc, m, n] and at[p, kc, n, m], h = kc*128 + p
        at = A_sh.transpose(1, 0, 2).reshape(KC, 128, NL, M)
        atnm = np.ascontiguousarray(at.transpose(1, 0, 2, 3)).astype(BF16)
        atmn = np.ascontiguousarray(
            at.transpose(1, 0, 3, 2)).astype(BF16)
        h0_sh = h0[ns]                                    # (NL, H)
        hh0 = np.zeros((128, KC, 32), dtype=BF16)
        hh0[:, :, :NL] = (2.0 * h0_sh.T).reshape(KC, 128, NL) \
            .transpose(1, 0, 2).astype(BF16)
        s0 = np.ascontiguousarray(
            (2.0 * h0_sh.T).reshape(KC, 128, NL).transpose(1, 0, 2)
            .astype(np.float32))
        m = {
            "xs": xs_st, "atmn": atmn, "atnm": atnm,
            "wx": wxs, "wh": whs, "wa": was,
            "hh0": hh0, "s0": s0, "id32": id32, "ones_col": ones_col,
        }
        if np.any(b != 0):
            m["bvec"] = bvec
            m["ones_row"] = np.ones((1, 128), dtype=BF16)
        maps.append(m)
    return maps


def _get_nc(has_bias, t_steps=T):
    key = (has_bias, t_steps)
    if key not in _CACHE:
        _CACHE[key] = build(t_steps=t_steps, has_bias=has_bias)
    return _CACHE[key]


def _unstage_out(outT, t_steps):
    """outT (t, 128, KC*NL) bf16 -> (NL, t, H) f32; h = kc*128+p; *0.5."""
    o = np.asarray(outT).reshape(t_steps, 128, KC, NL)
    return (o.transpose(3, 0, 2, 1).reshape(NL, t_steps, H)
            .astype(np.float32) * 0.5)


def run_cores(x, A, Wx, Wh, Wattn, b, t_steps=T, trace=False):
    from concourse.bass_utils import run_bass_kernel_spmd
    maps = _stage_inputs(x, A, Wx, Wh, Wattn, b, t_steps=t_steps)
    has_bias = "bvec" in maps[0]
    nc = _get_nc(has_bias, t_steps)
    res = run_bass_kernel_spmd(nc, maps, list(range(NCORES)), trace=trace)
    out = np.concatenate(
        [_unstage_out(res.results[k]["outT"], t_steps)
         for k in range(NCORES)], axis=0)
    return out, res


def kernel(x, A, Wx, Wh, Wattn, b):
    x = np.asarray(x, dtype=np.float32)
    A = np.asarray(A, dtype=np.float32)
    out, _ = run_cores(x, A,
                       np.asarray(Wx, dtype=np.float32),
                       np.asarray(Wh, dtype=np.float32),
                       np.asarray(Wattn, dtype=np.float32),
                       np.asarray(b, dtype=np.float32))
    return out
